# revision 21
# baseline (speedup 1.0000x reference)
"""BitSSM fused kernel for 8 Trainium2 NeuronCores.

Strategy
--------
Data-parallel over tokens: B*S = 16384 tokens split into 8 shards of 2048.
All ops are token-local except the causal depthwise conv (K=4), whose
3-token left halo is precomputed on the host per shard.

PE does only the three GEMMs (each 512-token pass costs ~220ns; fp8
DoubleRow contracts 2 K-planes per pass):
  in_proj : 6 fp16 planes + planes 0,1 as one fp8 DR pair  (7 MMs / tile)
  x_proj  : fp8 DoubleRow over X8 = fp8(64*xc)        (16 planes -> 8 MMs)
  out_proj: fp16 moving t = fp16(xc*gate), t planes 0..3 as two fp8 DR
            pairs with wo rows pre-scaled by exactly 1/64  (14 MMs / tile)
The K=4 causal depthwise conv runs as a shifted multiply-accumulate
TS/TT tree on DVE (scalar_tensor_tensor is 1x-mode, and GpSimd tensor ops
are software-slow and starve DVE -- keep GpSimd idle); PSUM->SBUF staging
copies plus silu and the X8 quantization run on the Scalar engine. A burst
of tiny warm-up matmuls during the initial DMA wait brings the PE out of
the HAM 1.2GHz cold state before the real stream starts. X8 lives in
per-pair tiles and t in per-plane tiles so matmuls depend only on the
planes they read; a few A(1) in_proj blocks are hoisted before B(0) and
C(0) blocks are interleaved into A(1) to keep the PE stream gap-free.

Phases per core (token halves H=1024; out_proj is token-local so phase C
runs per half too, which keeps SBUF small):
  A(h): in_proj -> (scalar copy to xi) -> DVE conv -> silu -> xc
        -> X8 (fp8, for x_proj only)
  B(h): x_proj -> gate = sigmoid(s_x/64 * psum + bx) -> t = fp16(xc*g)
  C(h): out_proj over t; out = Identity(s_out*psum + bo)
"""

import sys

if '/opt/trn_rl_repo' not in sys.path:
    sys.path.insert(0, '/opt/trn_rl_repo')

import numpy as np
import ml_dtypes

D_MODEL, D_STATE, D_INNER = 1024, 16, 2048
EPS = 1e-5
B, S = 4, 4096
N_CORES = 8
T = (B * S) // N_CORES          # tokens per core (2048)
H = T // 2                      # tokens per phase half (1024)
W = 512                         # psum tile width (tokens)
KI = D_MODEL // 128             # 8 contraction planes for in_proj
KF = KI - 2                     # fp16 planes (planes 0,1 ride fp8 DR)
KC = D_INNER // 128             # 16 contraction planes for x/out_proj
CT = D_INNER // 128             # 16 channel planes of d_inner
DT = D_MODEL // 128             # 8 channel planes of d_model
SC = 64.0                       # fp8 scale for xc

_BUILD_CACHE = {}


def _build(s_x: float, s_out: float):
    import concourse.tile as tile
    from concourse import bacc, mybir

    nc = bacc.Bacc("TRN2", target_bir_lowering=False, debug=False)
    f32 = mybir.dt.float32
    fp16 = mybir.dt.float16
    fp8 = mybir.dt.float8e4
    AF = mybir.ActivationFunctionType
    ALU = mybir.AluOpType
    DR = mybir.MatmulPerfMode.DoubleRow

    x16_d = nc.dram_tensor("x16", [128, KF, T], fp16, kind="ExternalInput")
    x8p_d = nc.dram_tensor("x8p", [128, 2, T], fp8, kind="ExternalInput")
    wi_d = nc.dram_tensor("wi", [128, KI * D_INNER], fp8, kind="ExternalInput")
    wx_d = nc.dram_tensor("wx", [128, KC * D_INNER], fp8, kind="ExternalInput")
    wo_d = nc.dram_tensor("wo", [128, KC * D_MODEL], fp8, kind="ExternalInput")
    wc_d = nc.dram_tensor("wc", [128, CT * 4], f32, kind="ExternalInput")
    bc_d = nc.dram_tensor("bc", [128, CT], f32, kind="ExternalInput")
    bx_d = nc.dram_tensor("bx", [128, CT], f32, kind="ExternalInput")
    bo_d = nc.dram_tensor("bo", [128, DT], f32, kind="ExternalInput")
    h0_d = nc.dram_tensor("h0", [128, CT * 3], f32, kind="ExternalInput")
    out_d = nc.dram_tensor("out", [128, DT * T], fp16, kind="ExternalOutput")

    with tile.TileContext(nc) as tc:
        with (
            tc.tile_pool(name="wx", bufs=1) as wxpool,
            tc.tile_pool(name="x8", bufs=1) as x8pool,
            tc.tile_pool(name="consts", bufs=1) as cpool,
            tc.tile_pool(name="ps", bufs=8, space="PSUM") as pspool,
        ):
            wx_t = wxpool.tile([128, KC, D_INNER], fp8, name="wx_t")
            wo_t = wxpool.tile([128, KC, D_MODEL], fp8, name="wo_t")
            # per-pair X8 tiles and per-plane t tiles so consumers depend
            # only on the planes they read (fine-grained pipelining)
            X8_p = [x8pool.tile([128, 2, H], fp8, name=f"X8_{k}")
                    for k in range(KC // 2)]
            t_p = [x8pool.tile([128, H], fp16, name=f"t_{c}")
                   for c in range(CT)]
            t8p = x8pool.tile([128, 4, H], fp8, name="t8p")

            # PE warm-up: small matmuls with no DMA dependency so the HAM
            # clock-gate opens while the first input DMAs are in flight.
            warm = cpool.tile([128, 128], fp16, name="warm")
            nc.vector.memset(warm[:], 0.0)
            ps_w = pspool.tile([128, 128], f32, tag="ps", name="ps_warm")
            for _ in range(20):
                nc.tensor.matmul(ps_w[:], warm[:], warm[:],
                                 start=True, stop=True)

            with (
                tc.tile_pool(name="xin", bufs=1) as xinpool,
                tc.tile_pool(name="wi", bufs=1) as wipool,
                tc.tile_pool(name="xi", bufs=4) as xipool,
                tc.tile_pool(name="acc", bufs=2) as accpool,
                tc.tile_pool(name="tap", bufs=2) as tappool,
                tc.tile_pool(name="xc", bufs=1) as xcpool,
                tc.tile_pool(name="gate", bufs=2) as gatepool,
                tc.tile_pool(name="out", bufs=2) as opool,
            ):
                wi_t = wipool.tile([128, CT, KI, 128], fp8, name="wi_t")
                wc_t = cpool.tile([128, CT * 4], f32, name="wc_t")
                bc_t = cpool.tile([128, CT], f32, name="bc_t")
                bx_t = cpool.tile([128, CT], f32, name="bx_t")
                bo_t = cpool.tile([128, DT], f32, name="bo_t")
                h0_t = cpool.tile([128, CT * 3], f32, name="h0_t")
                halo1 = cpool.tile([128, CT * 3], fp16, name="halo1")
                x16_t = [xinpool.tile([128, KF, H], fp16, name=f"x16_{h}")
                         for h in range(2)]
                x8p_t = xinpool.tile([128, 2, T], fp8, name="x8p_t")

                # critical-path-first DMA order: plane 0 of x and weights
                # unblock the first matmuls; bulk/late tensors follow.
                # two parallel DGE queues for the critical head: x16
                # planes on SP, wi planes on Activation; bulk loads queue
                # behind the critical ones on SP
                CW = KI * 128                    # wi cols per ct block
                # trigger order == consumption order: ct0 needs x16 plane 0
                # and its own wi slice first; everything else streams behind
                # tiny first transfer absorbs the DMA cold-start cost
                nc.sync.dma_start(wc_t[:], wc_d[:, :])
                nc.sync.dma_start(x16_t[0][:, 0, :], x16_d[:, 0, 0:H])
                nc.sync.dma_start(wi_t[:, 0, :, :], wi_d[:, 0:CW])
                nc.sync.dma_start(x16_t[0][:, 1, :], x16_d[:, 1, 0:H])
                nc.sync.dma_start(wi_t[:, 1:4, :, :], wi_d[:, CW:4 * CW])
                nc.sync.dma_start(x16_t[0][:, 2, :], x16_d[:, 2, 0:H])
                nc.sync.dma_start(x16_t[0][:, 3, :], x16_d[:, 3, 0:H])
                nc.sync.dma_start(x16_t[0][:, 4, :], x16_d[:, 4, 0:H])
                nc.sync.dma_start(x16_t[0][:, 5, :], x16_d[:, 5, 0:H])
                nc.sync.dma_start(x8p_t[:], x8p_d[:, :, :])
                nc.sync.dma_start(bc_t[:], bc_d[:, :])
                nc.sync.dma_start(h0_t[:], h0_d[:, :])
                nc.sync.dma_start(wi_t[:, 4:10, :, :], wi_d[:, 4 * CW:10 * CW])
                nc.sync.dma_start(wi_t[:, 10:16, :, :],
                                  wi_d[:, 10 * CW:16 * CW])
                nc.sync.dma_start(x16_t[1][:], x16_d[:, :, H:2 * H])
                nc.sync.dma_start(bx_t[:], bx_d[:, :])
                nc.sync.dma_start(bo_t[:], bo_d[:, :])
                nc.sync.dma_start(wx_t[:], wx_d[:, :])
                nc.sync.dma_start(wo_t[:], wo_d[:, :])

                A_state = {}

                def phase_A(h, cts, filler=None, defer_all=False,
                            flush=True):
                    if h not in A_state:
                        A_state[h] = {
                            "xc": xcpool.tile([128, CT, H], fp16, tag="xc",
                                              name=f"xc_{h}"),
                            "xi": {}, "pend": []}
                    st = A_state[h]
                    xc_t = st["xc"]
                    xi_map = st["xi"]

                    def stage_xi(ct, ps_in):
                        xi_t = xipool.tile([128, 4 + H], fp16, tag="xi",
                                           name=f"xi{ct}_{h}")
                        xi_map[ct] = xi_t
                        if h == 0:
                            nc.vector.tensor_copy(
                                xi_t[:, 1:4], h0_t[:, ct * 3:ct * 3 + 3])
                        else:
                            nc.vector.tensor_copy(
                                xi_t[:, 1:4], halo1[:, ct * 3:ct * 3 + 3])
                        for jh in range(2):
                            nc.scalar.activation(
                                xi_t[:, 4 + jh * W:4 + (jh + 1) * W],
                                ps_in[jh][:], AF.Identity, scale=1.0)
                        if h == 0:
                            nc.vector.tensor_copy(
                                halo1[:, ct * 3:ct * 3 + 3],
                                xi_t[:, 1 + H:4 + H])

                    def conv_group(ct):
                        xi_t = xi_map.pop(ct)
                        # K=4 causal depthwise conv as a balanced TS/TT
                        # tree on DVE (STT is 1x-mode; TS hits 4x, TT 2x);
                        # GpSimd is software-slow, keep it idle
                        a0 = accpool.tile([128, H], fp16, tag="a0",
                                          name=f"a0_{ct}_{h}")
                        a1 = accpool.tile([128, H], fp16, tag="a1",
                                          name=f"a1_{ct}_{h}")
                        a2 = accpool.tile([128, H], fp16, tag="a2",
                                          name=f"a2_{ct}_{h}")
                        tap_t = tappool.tile([128, H], fp16, tag="tap",
                                             name=f"tap{ct}_{h}")
                        nc.vector.tensor_scalar(
                            a0[:], xi_t[:, 1:1 + H],
                            wc_t[:, ct * 4:ct * 4 + 1], None, op0=ALU.mult)
                        nc.vector.tensor_scalar(
                            a1[:], xi_t[:, 2:2 + H],
                            wc_t[:, ct * 4 + 1:ct * 4 + 2], None, op0=ALU.mult)
                        nc.vector.tensor_scalar(
                            a2[:], xi_t[:, 3:3 + H],
                            wc_t[:, ct * 4 + 2:ct * 4 + 3], None, op0=ALU.mult)
                        nc.vector.tensor_scalar(
                            tap_t[:], xi_t[:, 4:4 + H],
                            wc_t[:, ct * 4 + 3:ct * 4 + 4], None, op0=ALU.mult)
                        nc.vector.tensor_tensor(
                            a0[:], a0[:], a1[:], op=ALU.add)
                        nc.vector.tensor_tensor(
                            tap_t[:], tap_t[:], a2[:], op=ALU.add)
                        nc.vector.tensor_tensor(
                            tap_t[:], tap_t[:], a0[:], op=ALU.add)
                        # silu and X8 quantization on the Scalar engine
                        nc.scalar.activation(
                            xc_t[:, ct, :], tap_t[:], AF.Silu,
                            bias=bc_t[:, ct:ct + 1], scale=1.0)
                        nc.scalar.activation(
                            X8_p[ct // 2][:, ct % 2, :], xc_t[:, ct, :],
                            AF.Identity, scale=SC)

                    pend = st["pend"]
                    for ct in cts:
                        ps_in = [pspool.tile([128, W], f32, tag="ps",
                                             name=f"psin{ct}_{jh}_{h}")
                                 for jh in range(2)]
                        for kt in range(KF):
                            wsl = wi_t[:, ct, 2 + kt, :]
                            for jh in range(2):
                                nc.tensor.matmul(
                                    ps_in[jh][:], wsl,
                                    x16_t[h][:, kt, jh * W:(jh + 1) * W],
                                    start=(kt == 0), stop=False)
                        wp = wi_t[:, ct, 0:2, :]
                        for jh in range(2):
                            nc.tensor.matmul(
                                ps_in[jh][:], wp,
                                x8p_t[:, :, h * H + jh * W:
                                      h * H + (jh + 1) * W],
                                start=False, stop=True, perf_mode=DR)
                        stage_xi(ct, ps_in)
                        pend.append(ct)
                        if not defer_all:
                            while len(pend) > 1:
                                conv_group(pend.pop(0))
                        if filler:
                            phase_C_block(*filler.pop(0))
                    if flush:
                        while pend:
                            conv_group(pend.pop(0))
                        while filler:
                            phase_C_block(*filler.pop(0))

                def phase_B(h):
                    xc_t = A_state[h]["xc"]
                    for c2 in range(CT):
                        ps_j = [pspool.tile([128, W], f32, tag="ps",
                                            name=f"psb{c2}_{jh}_{h}")
                                for jh in range(2)]
                        for kp in range(KC // 2):
                            wsl = wx_t[:, 2 * kp:2 * kp + 2,
                                       c2 * 128:(c2 + 1) * 128]
                            for jh in range(2):
                                nc.tensor.matmul(
                                    ps_j[jh][:], wsl,
                                    X8_p[kp][:, :, jh * W:(jh + 1) * W],
                                    start=(kp == 0), stop=(kp == KC // 2 - 1),
                                    perf_mode=DR)
                        gate_t = gatepool.tile([128, H], fp16, tag="g",
                                               name=f"g{c2}_{h}")
                        for jh in range(2):
                            nc.scalar.activation(
                                gate_t[:, jh * W:(jh + 1) * W], ps_j[jh][:],
                                AF.Sigmoid, bias=bx_t[:, c2:c2 + 1],
                                scale=s_x / SC)
                        nc.vector.tensor_tensor(
                            t_p[c2][:], xc_t[:, c2, :], gate_t[:],
                            op=ALU.mult)
                        if c2 < 4:
                            nc.vector.tensor_scalar_mul(
                                t8p[:, c2, :], t_p[c2][:], SC)

                def phase_C_block(h, dt, j):
                    ps_c = pspool.tile([128, W], f32, tag="ps",
                                       name=f"psc{dt}_{j}_{h}")
                    for c2 in range(4, CT):
                        wsl = wo_t[:, c2, dt * 128:(dt + 1) * 128]
                        nc.tensor.matmul(
                            ps_c[:], wsl,
                            t_p[c2][:, j * W:(j + 1) * W],
                            start=(c2 == 4), stop=False)
                    for kq in range(2):
                        nc.tensor.matmul(
                            ps_c[:],
                            wo_t[:, 2 * kq:2 * kq + 2, dt * 128:(dt + 1) * 128],
                            t8p[:, 2 * kq:2 * kq + 2, j * W:(j + 1) * W],
                            start=False, stop=(kq == 1), perf_mode=DR)
                    ot = opool.tile([128, W], fp16, tag="ot",
                                    name=f"ot{dt}_{j}_{h}")
                    nc.scalar.activation(
                        ot[:], ps_c[:], AF.Identity,
                        bias=bo_t[:, dt:dt + 1], scale=s_out)
                    nc.sync.dma_start(
                        out_d[:, dt * T + h * H + j * W:
                              dt * T + h * H + (j + 1) * W], ot[:])

                phase_A(0, range(CT))
                # two A(1) in_proj blocks (matmuls + staging only) keep the
                # PE fed while B(0) waits on the last X8 chains of A(0)
                phase_A(1, range(4), defer_all=True, flush=False)
                phase_B(0)
                # interleave C(0) blocks into A(1): C(0) matmuls are ready
                # (t planes of half 0 done) and fill PE holes while A(1)'s
                # DVE conv chains pace the in_proj stream
                phase_A(1, range(4, CT),
                        filler=[(0, dt, j)
                                for dt in range(DT) for j in range(2)])
                phase_B(1)
                for dt in range(DT):
                    for j in range(2):
                        phase_C_block(1, dt, j)

    nc.compile()
    return nc


def _quantize(w):
    s = np.float32(max(np.abs(w).mean(dtype=np.float64), EPS))
    return np.clip(np.round(w / s), -1.0, 1.0).astype(np.float32), s


def _plane_pack(a, nplanes, width):
    """[nplanes*128, width] -> [128, nplanes*width] with plane-major cols."""
    return np.ascontiguousarray(
        a.reshape(nplanes, 128, width).transpose(1, 0, 2).reshape(
            128, nplanes * width))


def kernel(x, w_in, b_in, w_conv, b_conv, w_x, b_x, w_out, b_out,
           _trace=False, _trace_kwargs=None):
    from concourse import bass_utils

    x = np.asarray(x, dtype=np.float32)
    w_in = np.asarray(w_in, dtype=np.float32)
    b_in = np.asarray(b_in, dtype=np.float32)
    w_conv = np.asarray(w_conv, dtype=np.float32)
    b_conv = np.asarray(b_conv, dtype=np.float32)
    w_x = np.asarray(w_x, dtype=np.float32)
    b_x = np.asarray(b_x, dtype=np.float32)
    w_out = np.asarray(w_out, dtype=np.float32)
    b_out = np.asarray(b_out, dtype=np.float32)

    # ---- host-side BitNet quantization (exact ternary) ----
    wq_in, s_in = _quantize(w_in)
    wq_x, s_x = _quantize(w_x)
    wq_out, s_out = _quantize(w_out)
    wq_in = wq_in[:D_INNER]           # res half unused downstream
    wq_x_d = wq_x[:D_INNER]           # only delta rows used

    fp8 = ml_dtypes.float8_e4m3
    fp16 = np.float16
    wi_pk = _plane_pack(np.ascontiguousarray(wq_in.T), KI, D_INNER).astype(fp8)
    wi_pk = np.ascontiguousarray(
        wi_pk.reshape(128, KI, CT, 128).transpose(0, 2, 1, 3).reshape(
            128, CT * KI * 128))
    wx_pk = _plane_pack(np.ascontiguousarray(wq_x_d.T), KC,
                        D_INNER).astype(fp8)
    wo_sc = np.ascontiguousarray(wq_out.T).copy()
    wo_sc[0:512, :] /= SC            # t planes 0..3 carried as fp8(SC*t)
    wo_pk = _plane_pack(wo_sc, KC, D_MODEL).astype(fp8)

    # conv taps as per-partition scalars (DVE/GpSimd shift-mult-accumulate)
    wc = (s_in * w_conv[:, 0, :]).astype(np.float32)             # [D_INNER, 4]
    wc_pk = np.ascontiguousarray(
        wc.reshape(CT, 128, 4).transpose(1, 0, 2).reshape(128, CT * 4))

    bc = (b_in[:D_INNER] * w_conv[:, 0, :].sum(axis=1)
          + b_conv).astype(np.float32)
    bc_pk = _plane_pack(bc, CT, 1)
    bx_pk = _plane_pack(b_x[:D_INNER].astype(np.float32), CT, 1)
    bo_pk = _plane_pack(b_out.astype(np.float32), DT, 1)

    # ---- shard inputs: x^T; dims 0..255 as an fp8 DR pair, rest fp16 ----
    x_flat = x.reshape(B * S, D_MODEL)
    xT = np.ascontiguousarray(x_flat.T)                   # [D_MODEL, B*S] f32
    xT16 = xT[256:].astype(fp16)                          # planes 2..7
    xT8 = xT[:256].astype(fp8)                            # planes 0,1

    # raw in_proj value that makes x_inner == 0 (sequence-start padding)
    pad_raw = (-b_in[:D_INNER] / s_in).astype(np.float32)

    in_maps = []
    for c in range(N_CORES):
        t0 = c * T
        x16 = _plane_pack(xT16[:, t0:t0 + T], KF, T).reshape(128, KF, T)
        x8p = np.ascontiguousarray(
            xT8[:, t0:t0 + T].reshape(2, 128, T).transpose(1, 0, 2))
        if t0 % S == 0:
            h0 = np.repeat(pad_raw[:, None], 3, axis=1)   # [D_INNER, 3]
        else:
            h0 = wq_in @ x_flat[t0 - 3:t0].T              # [D_INNER, 3]
        h0_pk = _plane_pack(h0.astype(np.float32), CT, 3)
        in_maps.append({
            "x16": x16, "x8p": x8p, "wi": wi_pk, "wx": wx_pk, "wo": wo_pk,
            "wc": wc_pk, "bc": bc_pk, "bx": bx_pk,
            "bo": bo_pk, "h0": h0_pk,
        })

    key = (float(s_x), float(s_out))
    if key not in _BUILD_CACHE:
        _BUILD_CACHE[key] = _build(float(s_x), float(s_out))
    nc = _BUILD_CACHE[key]

    kwargs = {}
    if _trace:
        kwargs["trace"] = True
        if _trace_kwargs:
            kwargs.update(_trace_kwargs)
    res = bass_utils.run_bass_kernel_spmd(
        nc, in_maps, core_ids=list(range(N_CORES)), **kwargs)
    kernel.last_results = res

    outs = []
    for c in range(N_CORES):
        arr = np.asarray(res.results[c]["out"]).astype(np.float32)
        outs.append(arr.reshape(128, DT, T).transpose(1, 0, 2).reshape(
            D_MODEL, T))
    full = np.concatenate(outs, axis=1)                   # [D_MODEL, B*S]
    return np.ascontiguousarray(full.T).reshape(B, S, D_MODEL).astype(
        np.float32)


# revision 23
# speedup vs baseline: 1.0070x; 1.0070x over previous
"""BitSSM fused kernel for 8 Trainium2 NeuronCores.

Strategy
--------
Data-parallel over tokens: B*S = 16384 tokens split into 8 shards of 2048.
All ops are token-local except the causal depthwise conv (K=4), whose
3-token left halo is precomputed on the host per shard.

PE does only the three GEMMs (each 512-token pass costs ~220ns; fp8
DoubleRow contracts 2 K-planes per pass):
  in_proj : 6 fp16 planes + planes 0,1 as one fp8 DR pair  (7 MMs / tile)
  x_proj  : fp8 DoubleRow over X8 = fp8(64*xc)        (16 planes -> 8 MMs)
  out_proj: fp16 moving t = fp16(xc*gate), t planes 0..3 as two fp8 DR
            pairs with wo rows pre-scaled by exactly 1/64  (14 MMs / tile)
The K=4 causal depthwise conv runs as a shifted multiply-accumulate
TS/TT tree on DVE (scalar_tensor_tensor is 1x-mode, and GpSimd tensor ops
are software-slow and starve DVE -- keep GpSimd idle); PSUM->SBUF staging
copies plus silu and the X8 quantization run on the Scalar engine. A burst
of tiny warm-up matmuls during the initial DMA wait brings the PE out of
the HAM 1.2GHz cold state before the real stream starts. X8 lives in
per-pair tiles and t in per-plane tiles so matmuls depend only on the
planes they read; a few A(1) in_proj blocks are hoisted before B(0) and
C(0) blocks are interleaved into A(1) to keep the PE stream gap-free.

Phases per core (token halves H=1024; out_proj is token-local so phase C
runs per half too, which keeps SBUF small):
  A(h): in_proj -> (scalar copy to xi) -> DVE conv -> silu -> xc
        -> X8 (fp8, for x_proj only)
  B(h): x_proj -> gate = sigmoid(s_x/64 * psum + bx) -> t = fp16(xc*g)
  C(h): out_proj over t; out = Identity(s_out*psum + bo)
"""

import sys

if '/opt/trn_rl_repo' not in sys.path:
    sys.path.insert(0, '/opt/trn_rl_repo')

import numpy as np
import ml_dtypes

D_MODEL, D_STATE, D_INNER = 1024, 16, 2048
EPS = 1e-5
B, S = 4, 4096
N_CORES = 8
T = (B * S) // N_CORES          # tokens per core (2048)
H = T // 2                      # tokens per phase half (1024)
W = 512                         # psum tile width (tokens)
KI = D_MODEL // 128             # 8 contraction planes for in_proj
KF = KI - 2                     # fp16 planes (planes 0,1 ride fp8 DR)
KC = D_INNER // 128             # 16 contraction planes for x/out_proj
CT = D_INNER // 128             # 16 channel planes of d_inner
DT = D_MODEL // 128             # 8 channel planes of d_model
SC = 64.0                       # fp8 scale for xc

_BUILD_CACHE = {}


def _build(s_x: float, s_out: float):
    import concourse.tile as tile
    from concourse import bacc, mybir

    nc = bacc.Bacc("TRN2", target_bir_lowering=False, debug=False)
    f32 = mybir.dt.float32
    fp16 = mybir.dt.float16
    fp8 = mybir.dt.float8e4
    AF = mybir.ActivationFunctionType
    ALU = mybir.AluOpType
    DR = mybir.MatmulPerfMode.DoubleRow

    x16_d = nc.dram_tensor("x16", [128, KF, T], fp16, kind="ExternalInput")
    x8p_d = nc.dram_tensor("x8p", [128, 2, T], fp8, kind="ExternalInput")
    wi_d = nc.dram_tensor("wi", [128, KI * D_INNER], fp8, kind="ExternalInput")
    wx_d = nc.dram_tensor("wx", [128, KC * D_INNER], fp8, kind="ExternalInput")
    wo_d = nc.dram_tensor("wo", [128, KC * D_MODEL], fp8, kind="ExternalInput")
    wc_d = nc.dram_tensor("wc", [128, CT * 4], f32, kind="ExternalInput")
    bc_d = nc.dram_tensor("bc", [128, CT], f32, kind="ExternalInput")
    bx_d = nc.dram_tensor("bx", [128, CT], f32, kind="ExternalInput")
    bo_d = nc.dram_tensor("bo", [128, DT], f32, kind="ExternalInput")
    h0_d = nc.dram_tensor("h0", [128, CT * 3], f32, kind="ExternalInput")
    out_d = nc.dram_tensor("out", [128, DT * T], fp16, kind="ExternalOutput")

    with tile.TileContext(nc) as tc:
        with (
            tc.tile_pool(name="wx", bufs=1) as wxpool,
            tc.tile_pool(name="x8", bufs=1) as x8pool,
            tc.tile_pool(name="consts", bufs=1) as cpool,
            tc.tile_pool(name="ps", bufs=8, space="PSUM") as pspool,
        ):
            wx_t = wxpool.tile([128, KC, D_INNER], fp8, name="wx_t")
            wo_t = wxpool.tile([128, KC, D_MODEL], fp8, name="wo_t")
            # per-pair X8 tiles and per-plane t tiles so consumers depend
            # only on the planes they read (fine-grained pipelining)
            X8_p = [x8pool.tile([128, 2, H], fp8, name=f"X8_{k}")
                    for k in range(KC // 2)]
            t_p = [x8pool.tile([128, H], fp16, name=f"t_{c}")
                   for c in range(CT)]
            t8p = x8pool.tile([128, 4, H], fp8, name="t8p")

            # PE warm-up: small matmuls with no DMA dependency so the HAM
            # clock-gate opens while the first input DMAs are in flight.
            warm = cpool.tile([128, 128], fp16, name="warm")
            nc.vector.memset(warm[:], 0.0)
            ps_w = pspool.tile([128, 128], f32, tag="ps", name="ps_warm")
            for _ in range(16):
                nc.tensor.matmul(ps_w[:], warm[:], warm[:],
                                 start=True, stop=True)

            with (
                tc.tile_pool(name="xin", bufs=1) as xinpool,
                tc.tile_pool(name="wi", bufs=1) as wipool,
                tc.tile_pool(name="xi", bufs=4) as xipool,
                tc.tile_pool(name="acc", bufs=2) as accpool,
                tc.tile_pool(name="tap", bufs=2) as tappool,
                tc.tile_pool(name="xc", bufs=1) as xcpool,
                tc.tile_pool(name="gate", bufs=2) as gatepool,
                tc.tile_pool(name="out", bufs=2) as opool,
            ):
                wi_t = wipool.tile([128, CT, KI, 128], fp8, name="wi_t")
                wc_t = cpool.tile([128, CT * 4], f32, name="wc_t")
                bc_t = cpool.tile([128, CT], f32, name="bc_t")
                bx_t = cpool.tile([128, CT], f32, name="bx_t")
                bo_t = cpool.tile([128, DT], f32, name="bo_t")
                h0_t = cpool.tile([128, CT * 3], f32, name="h0_t")
                halo1 = cpool.tile([128, CT * 3], fp16, name="halo1")
                x16_t = [xinpool.tile([128, KF, H], fp16, name=f"x16_{h}")
                         for h in range(2)]
                x8p_t = xinpool.tile([128, 2, T], fp8, name="x8p_t")

                # critical-path-first DMA order: plane 0 of x and weights
                # unblock the first matmuls; bulk/late tensors follow.
                # two parallel DGE queues for the critical head: x16
                # planes on SP, wi planes on Activation; bulk loads queue
                # behind the critical ones on SP
                CW = KI * 128                    # wi cols per ct block
                # trigger order == consumption order: ct0 needs x16 plane 0
                # and its own wi slice first; everything else streams behind
                nc.sync.dma_start(x16_t[0][:, 0, :], x16_d[:, 0, 0:H])
                nc.sync.dma_start(wi_t[:, 0, :, :], wi_d[:, 0:CW])
                nc.sync.dma_start(x16_t[0][:, 1, :], x16_d[:, 1, 0:H])
                nc.sync.dma_start(wi_t[:, 1:4, :, :], wi_d[:, CW:4 * CW])
                nc.sync.dma_start(x16_t[0][:, 2, :], x16_d[:, 2, 0:H])
                nc.sync.dma_start(x16_t[0][:, 3, :], x16_d[:, 3, 0:H])
                nc.sync.dma_start(x16_t[0][:, 4, :], x16_d[:, 4, 0:H])
                nc.sync.dma_start(x16_t[0][:, 5, :], x16_d[:, 5, 0:H])
                nc.sync.dma_start(x8p_t[:], x8p_d[:, :, :])
                nc.sync.dma_start(wc_t[:], wc_d[:, :])
                nc.sync.dma_start(bc_t[:], bc_d[:, :])
                nc.sync.dma_start(h0_t[:], h0_d[:, :])
                nc.sync.dma_start(wi_t[:, 4:10, :, :], wi_d[:, 4 * CW:10 * CW])
                nc.sync.dma_start(wi_t[:, 10:16, :, :],
                                  wi_d[:, 10 * CW:16 * CW])
                nc.sync.dma_start(x16_t[1][:], x16_d[:, :, H:2 * H])
                nc.sync.dma_start(bx_t[:], bx_d[:, :])
                nc.sync.dma_start(bo_t[:], bo_d[:, :])
                nc.sync.dma_start(wx_t[:], wx_d[:, :])
                nc.sync.dma_start(wo_t[:], wo_d[:, :])

                A_state = {}

                def phase_A(h, cts, filler=None, defer_all=False,
                            flush=True):
                    if h not in A_state:
                        A_state[h] = {
                            "xc": xcpool.tile([128, CT, H], fp16, tag="xc",
                                              name=f"xc_{h}"),
                            "xi": {}, "pend": []}
                    st = A_state[h]
                    xc_t = st["xc"]
                    xi_map = st["xi"]

                    def stage_xi(ct, ps_in):
                        xi_t = xipool.tile([128, 4 + H], fp16, tag="xi",
                                           name=f"xi{ct}_{h}")
                        xi_map[ct] = xi_t
                        if h == 0:
                            nc.vector.tensor_copy(
                                xi_t[:, 1:4], h0_t[:, ct * 3:ct * 3 + 3])
                        else:
                            nc.vector.tensor_copy(
                                xi_t[:, 1:4], halo1[:, ct * 3:ct * 3 + 3])
                        for jh in range(2):
                            nc.scalar.activation(
                                xi_t[:, 4 + jh * W:4 + (jh + 1) * W],
                                ps_in[jh][:], AF.Identity, scale=1.0)
                        if h == 0:
                            nc.vector.tensor_copy(
                                halo1[:, ct * 3:ct * 3 + 3],
                                xi_t[:, 1 + H:4 + H])

                    def conv_group(ct):
                        xi_t = xi_map.pop(ct)
                        # K=4 causal depthwise conv as a balanced TS/TT
                        # tree on DVE (STT is 1x-mode; TS hits 4x, TT 2x);
                        # GpSimd is software-slow, keep it idle
                        a0 = accpool.tile([128, H], fp16, tag="a0",
                                          name=f"a0_{ct}_{h}")
                        a1 = accpool.tile([128, H], fp16, tag="a1",
                                          name=f"a1_{ct}_{h}")
                        a2 = accpool.tile([128, H], fp16, tag="a2",
                                          name=f"a2_{ct}_{h}")
                        tap_t = tappool.tile([128, H], fp16, tag="tap",
                                             name=f"tap{ct}_{h}")
                        nc.vector.tensor_scalar(
                            a0[:], xi_t[:, 1:1 + H],
                            wc_t[:, ct * 4:ct * 4 + 1], None, op0=ALU.mult)
                        nc.vector.tensor_scalar(
                            a1[:], xi_t[:, 2:2 + H],
                            wc_t[:, ct * 4 + 1:ct * 4 + 2], None, op0=ALU.mult)
                        nc.vector.tensor_scalar(
                            a2[:], xi_t[:, 3:3 + H],
                            wc_t[:, ct * 4 + 2:ct * 4 + 3], None, op0=ALU.mult)
                        nc.vector.tensor_scalar(
                            tap_t[:], xi_t[:, 4:4 + H],
                            wc_t[:, ct * 4 + 3:ct * 4 + 4], None, op0=ALU.mult)
                        nc.vector.tensor_tensor(
                            a0[:], a0[:], a1[:], op=ALU.add)
                        nc.vector.tensor_tensor(
                            tap_t[:], tap_t[:], a2[:], op=ALU.add)
                        nc.vector.tensor_tensor(
                            tap_t[:], tap_t[:], a0[:], op=ALU.add)
                        # silu and X8 quantization on the Scalar engine
                        nc.scalar.activation(
                            xc_t[:, ct, :], tap_t[:], AF.Silu,
                            bias=bc_t[:, ct:ct + 1], scale=1.0)
                        nc.scalar.activation(
                            X8_p[ct // 2][:, ct % 2, :], xc_t[:, ct, :],
                            AF.Identity, scale=SC)

                    pend = st["pend"]
                    for ct in cts:
                        if h == 0 and ct == 0:
                            # plane-interleave ct0/ct1 so consumption rate
                            # matches the head DMA plane-arrival rate
                            psp = [[pspool.tile([128, W], f32, tag="ps",
                                                name=f"psin{c}_{jh}_0")
                                    for jh in range(2)] for c in range(2)]
                            for kt in range(KF):
                                for c in range(2):
                                    wsl = wi_t[:, c, 2 + kt, :]
                                    for jh in range(2):
                                        nc.tensor.matmul(
                                            psp[c][jh][:], wsl,
                                            x16_t[0][:, kt,
                                                     jh * W:(jh + 1) * W],
                                            start=(kt == 0), stop=False)
                            for c in range(2):
                                wp = wi_t[:, c, 0:2, :]
                                for jh in range(2):
                                    nc.tensor.matmul(
                                        psp[c][jh][:], wp,
                                        x8p_t[:, :, jh * W:(jh + 1) * W],
                                        start=False, stop=True, perf_mode=DR)
                                stage_xi(c, psp[c])
                                pend.append(c)
                            continue
                        if h == 0 and ct == 1:
                            continue
                        ps_in = [pspool.tile([128, W], f32, tag="ps",
                                             name=f"psin{ct}_{jh}_{h}")
                                 for jh in range(2)]
                        for kt in range(KF):
                            wsl = wi_t[:, ct, 2 + kt, :]
                            for jh in range(2):
                                nc.tensor.matmul(
                                    ps_in[jh][:], wsl,
                                    x16_t[h][:, kt, jh * W:(jh + 1) * W],
                                    start=(kt == 0), stop=False)
                        wp = wi_t[:, ct, 0:2, :]
                        for jh in range(2):
                            nc.tensor.matmul(
                                ps_in[jh][:], wp,
                                x8p_t[:, :, h * H + jh * W:
                                      h * H + (jh + 1) * W],
                                start=False, stop=True, perf_mode=DR)
                        stage_xi(ct, ps_in)
                        pend.append(ct)
                        if not defer_all:
                            while len(pend) > 1:
                                conv_group(pend.pop(0))
                        if filler:
                            phase_C_block(*filler.pop(0))
                    if flush:
                        while pend:
                            conv_group(pend.pop(0))
                        while filler:
                            phase_C_block(*filler.pop(0))

                def phase_B(h):
                    xc_t = A_state[h]["xc"]
                    for c2 in range(CT):
                        ps_j = [pspool.tile([128, W], f32, tag="ps",
                                            name=f"psb{c2}_{jh}_{h}")
                                for jh in range(2)]
                        for kp in range(KC // 2):
                            wsl = wx_t[:, 2 * kp:2 * kp + 2,
                                       c2 * 128:(c2 + 1) * 128]
                            for jh in range(2):
                                nc.tensor.matmul(
                                    ps_j[jh][:], wsl,
                                    X8_p[kp][:, :, jh * W:(jh + 1) * W],
                                    start=(kp == 0), stop=(kp == KC // 2 - 1),
                                    perf_mode=DR)
                        gate_t = gatepool.tile([128, H], fp16, tag="g",
                                               name=f"g{c2}_{h}")
                        for jh in range(2):
                            nc.scalar.activation(
                                gate_t[:, jh * W:(jh + 1) * W], ps_j[jh][:],
                                AF.Sigmoid, bias=bx_t[:, c2:c2 + 1],
                                scale=s_x / SC)
                        nc.vector.tensor_tensor(
                            t_p[c2][:], xc_t[:, c2, :], gate_t[:],
                            op=ALU.mult)
                        if c2 < 4:
                            nc.vector.tensor_scalar_mul(
                                t8p[:, c2, :], t_p[c2][:], SC)

                def phase_C_block(h, dt, j):
                    ps_c = pspool.tile([128, W], f32, tag="ps",
                                       name=f"psc{dt}_{j}_{h}")
                    for c2 in range(4, CT):
                        wsl = wo_t[:, c2, dt * 128:(dt + 1) * 128]
                        nc.tensor.matmul(
                            ps_c[:], wsl,
                            t_p[c2][:, j * W:(j + 1) * W],
                            start=(c2 == 4), stop=False)
                    for kq in range(2):
                        nc.tensor.matmul(
                            ps_c[:],
                            wo_t[:, 2 * kq:2 * kq + 2, dt * 128:(dt + 1) * 128],
                            t8p[:, 2 * kq:2 * kq + 2, j * W:(j + 1) * W],
                            start=False, stop=(kq == 1), perf_mode=DR)
                    ot = opool.tile([128, W], fp16, tag="ot",
                                    name=f"ot{dt}_{j}_{h}")
                    nc.scalar.activation(
                        ot[:], ps_c[:], AF.Identity,
                        bias=bo_t[:, dt:dt + 1], scale=s_out)
                    nc.sync.dma_start(
                        out_d[:, dt * T + h * H + j * W:
                              dt * T + h * H + (j + 1) * W], ot[:])

                phase_A(0, range(CT))
                # two A(1) in_proj blocks (matmuls + staging only) keep the
                # PE fed while B(0) waits on the last X8 chains of A(0)
                phase_A(1, range(4), defer_all=True, flush=False)
                phase_B(0)
                # interleave C(0) blocks into A(1): C(0) matmuls are ready
                # (t planes of half 0 done) and fill PE holes while A(1)'s
                # DVE conv chains pace the in_proj stream
                phase_A(1, range(4, CT),
                        filler=[(0, dt, j)
                                for dt in range(DT) for j in range(2)])
                phase_B(1)
                for dt in range(DT):
                    for j in range(2):
                        phase_C_block(1, dt, j)

    nc.compile()
    return nc


def _quantize(w):
    s = np.float32(max(np.abs(w).mean(dtype=np.float64), EPS))
    return np.clip(np.round(w / s), -1.0, 1.0).astype(np.float32), s


def _plane_pack(a, nplanes, width):
    """[nplanes*128, width] -> [128, nplanes*width] with plane-major cols."""
    return np.ascontiguousarray(
        a.reshape(nplanes, 128, width).transpose(1, 0, 2).reshape(
            128, nplanes * width))


def kernel(x, w_in, b_in, w_conv, b_conv, w_x, b_x, w_out, b_out,
           _trace=False, _trace_kwargs=None):
    from concourse import bass_utils

    x = np.asarray(x, dtype=np.float32)
    w_in = np.asarray(w_in, dtype=np.float32)
    b_in = np.asarray(b_in, dtype=np.float32)
    w_conv = np.asarray(w_conv, dtype=np.float32)
    b_conv = np.asarray(b_conv, dtype=np.float32)
    w_x = np.asarray(w_x, dtype=np.float32)
    b_x = np.asarray(b_x, dtype=np.float32)
    w_out = np.asarray(w_out, dtype=np.float32)
    b_out = np.asarray(b_out, dtype=np.float32)

    # ---- host-side BitNet quantization (exact ternary) ----
    wq_in, s_in = _quantize(w_in)
    wq_x, s_x = _quantize(w_x)
    wq_out, s_out = _quantize(w_out)
    wq_in = wq_in[:D_INNER]           # res half unused downstream
    wq_x_d = wq_x[:D_INNER]           # only delta rows used

    fp8 = ml_dtypes.float8_e4m3
    fp16 = np.float16
    wi_pk = _plane_pack(np.ascontiguousarray(wq_in.T), KI, D_INNER).astype(fp8)
    wi_pk = np.ascontiguousarray(
        wi_pk.reshape(128, KI, CT, 128).transpose(0, 2, 1, 3).reshape(
            128, CT * KI * 128))
    wx_pk = _plane_pack(np.ascontiguousarray(wq_x_d.T), KC,
                        D_INNER).astype(fp8)
    wo_sc = np.ascontiguousarray(wq_out.T).copy()
    wo_sc[0:512, :] /= SC            # t planes 0..3 carried as fp8(SC*t)
    wo_pk = _plane_pack(wo_sc, KC, D_MODEL).astype(fp8)

    # conv taps as per-partition scalars (DVE/GpSimd shift-mult-accumulate)
    wc = (s_in * w_conv[:, 0, :]).astype(np.float32)             # [D_INNER, 4]
    wc_pk = np.ascontiguousarray(
        wc.reshape(CT, 128, 4).transpose(1, 0, 2).reshape(128, CT * 4))

    bc = (b_in[:D_INNER] * w_conv[:, 0, :].sum(axis=1)
          + b_conv).astype(np.float32)
    bc_pk = _plane_pack(bc, CT, 1)
    bx_pk = _plane_pack(b_x[:D_INNER].astype(np.float32), CT, 1)
    bo_pk = _plane_pack(b_out.astype(np.float32), DT, 1)

    # ---- shard inputs: x^T; dims 0..255 as an fp8 DR pair, rest fp16 ----
    x_flat = x.reshape(B * S, D_MODEL)
    xT = np.ascontiguousarray(x_flat.T)                   # [D_MODEL, B*S] f32
    xT16 = xT[256:].astype(fp16)                          # planes 2..7
    xT8 = xT[:256].astype(fp8)                            # planes 0,1

    # raw in_proj value that makes x_inner == 0 (sequence-start padding)
    pad_raw = (-b_in[:D_INNER] / s_in).astype(np.float32)

    in_maps = []
    for c in range(N_CORES):
        t0 = c * T
        x16 = _plane_pack(xT16[:, t0:t0 + T], KF, T).reshape(128, KF, T)
        x8p = np.ascontiguousarray(
            xT8[:, t0:t0 + T].reshape(2, 128, T).transpose(1, 0, 2))
        if t0 % S == 0:
            h0 = np.repeat(pad_raw[:, None], 3, axis=1)   # [D_INNER, 3]
        else:
            h0 = wq_in @ x_flat[t0 - 3:t0].T              # [D_INNER, 3]
        h0_pk = _plane_pack(h0.astype(np.float32), CT, 3)
        in_maps.append({
            "x16": x16, "x8p": x8p, "wi": wi_pk, "wx": wx_pk, "wo": wo_pk,
            "wc": wc_pk, "bc": bc_pk, "bx": bx_pk,
            "bo": bo_pk, "h0": h0_pk,
        })

    key = (float(s_x), float(s_out))
    if key not in _BUILD_CACHE:
        _BUILD_CACHE[key] = _build(float(s_x), float(s_out))
    nc = _BUILD_CACHE[key]

    kwargs = {}
    if _trace:
        kwargs["trace"] = True
        if _trace_kwargs:
            kwargs.update(_trace_kwargs)
    res = bass_utils.run_bass_kernel_spmd(
        nc, in_maps, core_ids=list(range(N_CORES)), **kwargs)
    kernel.last_results = res

    outs = []
    for c in range(N_CORES):
        arr = np.asarray(res.results[c]["out"]).astype(np.float32)
        outs.append(arr.reshape(128, DT, T).transpose(1, 0, 2).reshape(
            D_MODEL, T))
    full = np.concatenate(outs, axis=1)                   # [D_MODEL, B*S]
    return np.ascontiguousarray(full.T).reshape(B, S, D_MODEL).astype(
        np.float32)
